# revision 18
# baseline (speedup 1.0000x reference)
"""AttentionWithBias (AlphaFold-style gated attention with pair bias) on 8 trn2 cores.

Sharding: core c handles batch b = c//4, query block qb = c%4 (128 queries).
Each core streams its [128, 512, 128] f32 bias slice from HBM ONCE, as a
host-side pre-transposed bf16 copy [d, k, q] (16 MB/core, ~47 us DMA) — half
the HBM traffic of the previous two-copy scheme.

Per key k the PE loads the [d=128, q=128] bias tile as stationary weights and
issues two matmuls:
  raw[q, 0:16] = tile^T @ wext      (cols 0..7 = g*Wb - c1/128, col 8 = 1/128)
  ss[q]        = sqtile^T @ ones    (sum of squares over d, out-free-size 1)
where sqtile = tile*tile is produced by one contiguous bf16 DVE multiply per
half-chunk (2x DVE mode).  This moves the LayerNorm variance reduction off
the vector/scalar engines (which were the bottleneck: a 67 us DVE add-tree +
35 us of ACT squares) onto the PE, where the reduction costs ~3 ns per key.

LayerNorm is folded into a post-matmul affine fixup as before:
  bias_h[r, h] = raw[r, h] * rinv_r   (+const_h, cancels in softmax)
  rinv = exp(-0.5*ln(var+eps)), var = ss/128 - mean^2, mean = raw[r, 8].

The key-side mask is folded into the S logits via a rank-1 PE matmul
(ones^T @ mask_row) accumulated into the same PSUM as q@k^T.  The softmax
denominator is folded into the PV matmuls as an extra ones-column (out col
256+h), so no separate DVE reduction is needed.  All fixup/exp tensors use a
[q, h, k] layout so every ACT write is innermost-contiguous.  Softmax runs
without max-subtraction (logits are O(10); masked lanes get -2e9 -> exp 0);
per-(q,h)-constant terms cancel in softmax so c2 and the query-side mask drop
out (fully-masked rows are zeroed by the final row mask).
"""

import sys

if "/opt/trn_rl_repo" not in sys.path:
    sys.path.insert(0, "/opt/trn_rl_repo")

from contextlib import ExitStack

import ml_dtypes
import numpy as np

import concourse.bacc as bacc
import concourse.bass as bass
import concourse.tile as tile
from concourse import mybir
from concourse.bass_utils import run_bass_kernel_spmd

BF16 = ml_dtypes.bfloat16
F32 = mybir.dt.float32
BF = mybir.dt.bfloat16
AF = mybir.ActivationFunctionType
OP = mybir.AluOpType

D_IN = 256
D_BIAS = 128
H = 8
DH = 32
B = 2
L = 512
SCALE = 1.0 / np.sqrt(DH)
QB = 128          # queries per core
KC = 64           # keys per streamed chunk
NCH = L // KC     # chunks
HC = 32           # keys per PSUM half
NEG = -2.0e9
EPS = 1e-5

_CACHE = {}


def _ap(base, off, dims):
    return bass.AP(tensor=base.tensor, offset=base.offset + off, ap=[list(base.ap[0])] + dims)


def _build():
    nc = bacc.Bacc("TRN2", target_bir_lowering=False, debug=False, num_devices=8)

    bias_tr = nc.declare_dram_parameter("bias_tr", [D_BIAS, L, QB], BF, isOutput=False)
    # x rows (4x128), x_q, key mask rows — one early DMA (stats + mask only)
    x_all = nc.declare_dram_parameter("x_all", [128, 7, D_IN], BF, isOutput=False)
    # host-transposed x: [din-chunk, 2, 512 batch rows + 128 q rows]
    x_allT = nc.declare_dram_parameter("x_allT", [128, 2, 640], BF, isOutput=False)
    # wq|wk|wv|wg|wo as [128, 2, 256] din-chunk groups; slot 10 = wext; slot 11 = I128
    wall = nc.declare_dram_parameter("wall", [128, 12, D_IN], BF, isOutput=False)
    rowm = nc.declare_dram_parameter("rowm", [128, 1], F32, isOutput=False)
    # per-projection row biases [1, 256] (ln_in_b folded through each W, + bg for gate)
    brows = nc.declare_dram_parameter("brows", [5, D_IN], BF, isOutput=False)

    out = nc.declare_dram_parameter("out", [QB, D_IN], F32, isOutput=True)

    with tile.TileContext(nc) as tc, ExitStack() as ctx:
        sing = ctx.enter_context(tc.tile_pool(name="sing", bufs=1))
        ldp = ctx.enter_context(tc.tile_pool(name="ldp", bufs=3))
        sqp = ctx.enter_context(tc.tile_pool(name="sqp", bufs=3))
        scr = ctx.enter_context(tc.tile_pool(name="scr", bufs=3))
        pvp = ctx.enter_context(tc.tile_pool(name="pvp", bufs=2))
        ps = ctx.enter_context(tc.tile_pool(name="ps", bufs=2, space="PSUM"))
        ps_raw = ctx.enter_context(tc.tile_pool(name="ps_raw", bufs=4, space="PSUM"))
        ssps = ctx.enter_context(tc.tile_pool(name="ssps", bufs=2, space="PSUM"))

        # ---------------- phase 0: batched early loads on SP ----------------
        xall_sb = sing.tile([128, 7, D_IN], BF)
        nc.sync.dma_start(out=xall_sb[:], in_=x_all[:, :, :])
        xallT_sb = sing.tile([128, 2, 640], BF)
        nc.sync.dma_start(out=xallT_sb[:], in_=x_allT[:, :, :])
        wall_sb = sing.tile([128, 12, D_IN], BF)
        nc.sync.dma_start(out=wall_sb[:], in_=wall[:, :, :])
        wext_sb = wall_sb[:, 10, 0:16]
        i128 = wall_sb[:, 11, 0:128]
        w_sb = {nm: wall_sb[:, 2 * i:2 * i + 2, :]
                for i, nm in enumerate(("q", "k", "v", "g", "o"))}
        mk_sb = xall_sb[:, 5:7, :]
        brow_sb = sing.tile([1, 5, D_IN], BF)
        nc.gpsimd.dma_start(out=brow_sb[:], in_=brows[None, :, :])
        ones_row = sing.tile([1, L], BF)
        nc.vector.memset(ones_row[:], 1.0)
        ones_col = sing.tile([128, 1], BF)
        nc.vector.memset(ones_col[:], 1.0)
        rowm_sb = sing.tile([128, 1], F32)
        nc.gpsimd.dma_start(out=rowm_sb[:], in_=rowm[:, :])
        eps_sb = sing.tile([128, 1], F32)
        nc.vector.memset(eps_sb[:], EPS)

        # ---- x LayerNorm, applied in transposed space ----
        # stats per row from the natural copy; then xnT = (xT - m) * rstd with
        # m/rstd broadcast along partitions via PE rank-1s (stat columns are
        # turned into rows by one PE transpose against I128).
        mv_all = sing.tile([128, 5, 2], F32)
        for r in range(5):
            st6 = scr.tile([128, 6], F32, tag="ln_st6")
            nc.vector.bn_stats(out=st6[:], in_=xall_sb[:, r, :])
            nc.vector.bn_aggr(out=mv_all[:, r, :], in_=st6[:])
        lnv5 = sing.tile([128, 5], F32)
        nc.scalar.activation(lnv5[:], _ap(mv_all[:], 1, [[2, 5]]), AF.Ln,
                             bias=eps_sb[:, 0:1])
        stat_bf = sing.tile([128, 2, 5], BF)
        nc.scalar.activation(stat_bf[:, 1, :], lnv5[:], AF.Exp, scale=-0.5)
        nc.vector.tensor_copy(stat_bf[:, 0, :], _ap(mv_all[:], 0, [[2, 5]]))
        statT = sing.tile([1, 1280], BF)
        for g in range(3):
            pstT = ps.tile([128, L], F32, tag="p0")
            n = min(4, 10 - g * 4)
            for j in range(n):
                stat, blk = divmod(g * 4 + j, 5)
                nc.tensor.matmul(pstT[0:1, j * 128:(j + 1) * 128],
                                 lhsT=stat_bf[:, stat, blk:blk + 1],
                                 rhs=i128, start=True, stop=True)
            nc.scalar.copy(statT[:, g * 512:g * 512 + n * 128],
                           pstT[0:1, 0:n * 128])

        xnT = sing.tile([128, 2, 640], BF)
        for blk in range(5):
            pbr = ps.tile([128, L], F32, tag="p0")
            nc.tensor.matmul(pbr[:, 0:128], lhsT=ones_row[:, 0:QB],
                             rhs=statT[:, blk * 128:(blk + 1) * 128],
                             start=True, stop=True)
            nc.tensor.matmul(pbr[:, 128:256], lhsT=ones_row[:, 0:QB],
                             rhs=statT[:, (5 + blk) * 128:(6 + blk) * 128],
                             start=True, stop=True)
            csl = slice(blk * 128, (blk + 1) * 128)
            tx = scr.tile([128, 2, 128], F32, tag="xnt_t")
            nc.vector.tensor_tensor(out=tx[:], in0=xallT_sb[:, :, csl],
                                    in1=_ap(pbr[:], 0, [[0, 2], [1, 128]]),
                                    op=OP.subtract)
            nc.vector.tensor_tensor(out=xnT[:, :, csl], in0=tx[:],
                                    in1=_ap(pbr[:], 128, [[0, 2], [1, 128]]),
                                    op=OP.mult)

        # ---- kT, qT ----
        kT = sing.tile([128, 2, L], BF)
        for h2 in range(2):
            pk = ps.tile([128, L], F32, tag="p0")
            nc.tensor.matmul(pk[:], lhsT=w_sb["k"][:, 0, h2 * 128:(h2 + 1) * 128],
                             rhs=xnT[:, 0, 0:L], start=True, stop=False)
            nc.tensor.matmul(pk[:], lhsT=w_sb["k"][:, 1, h2 * 128:(h2 + 1) * 128],
                             rhs=xnT[:, 1, 0:L], start=False, stop=False)
            nc.tensor.matmul(pk[:], lhsT=brow_sb[:, 1, h2 * 128:(h2 + 1) * 128],
                             rhs=ones_row[:], start=False, stop=True)
            nc.scalar.copy(kT[:, h2, :], pk[:])
        qT = sing.tile([128, 2, QB], BF)
        for h2 in range(2):
            pq = ps.tile([128, QB], F32, tag="p0")
            nc.tensor.matmul(pq[:], lhsT=w_sb["q"][:, 0, h2 * 128:(h2 + 1) * 128],
                             rhs=xnT[:, 0, 512:640], start=True, stop=False)
            nc.tensor.matmul(pq[:], lhsT=w_sb["q"][:, 1, h2 * 128:(h2 + 1) * 128],
                             rhs=xnT[:, 1, 512:640], start=False, stop=False)
            nc.tensor.matmul(pq[:], lhsT=brow_sb[:, 0, h2 * 128:(h2 + 1) * 128],
                             rhs=ones_row[:, 0:QB], start=False, stop=True)
            nc.scalar.copy(qT[:, h2, :], pq[:])

        # ---- v_ext (natural [k rows, h*(dh+1)]) with a per-head ones column
        #      so the PV matmul also accumulates the softmax denominator ----
        v_sb = sing.tile([128, 4, H * (DH + 1)], BF)
        nc.vector.memset(v_sb[:], 1.0)
        for r in range(4):
            pv = ps.tile([128, D_IN], F32, tag="p0")
            nc.tensor.matmul(pv[:], lhsT=xnT[:, 0, r * 128:(r + 1) * 128],
                             rhs=w_sb["v"][:, 0, :], start=True, stop=False)
            nc.tensor.matmul(pv[:], lhsT=xnT[:, 1, r * 128:(r + 1) * 128],
                             rhs=w_sb["v"][:, 1, :], start=False, stop=False)
            nc.tensor.matmul(pv[:], lhsT=ones_row[:, 0:128],
                             rhs=brow_sb[:, 2, :], start=False, stop=True)
            nc.scalar.copy(_ap(v_sb[:, r, :], 0, [[DH + 1, H], [1, DH]]),
                           pv[:].rearrange("p (h d) -> p h d", h=H))

        # ---- gate = sigmoid(xq @ Wg + bgate) ----
        gate_sb = sing.tile([128, D_IN], F32)
        pg = ps.tile([128, D_IN], F32, tag="p0")
        nc.tensor.matmul(pg[:], lhsT=xnT[:, 0, 512:640], rhs=w_sb["g"][:, 0, :],
                         start=True, stop=False)
        nc.tensor.matmul(pg[:], lhsT=xnT[:, 1, 512:640], rhs=w_sb["g"][:, 1, :],
                         start=False, stop=False)
        nc.tensor.matmul(pg[:], lhsT=ones_row[:, 0:128], rhs=brow_sb[:, 3, :],
                         start=False, stop=True)
        # sigmoid(x) = 1/(1+exp(-x)) — avoids loading the sigmoid ACT table set
        nc.scalar.activation(gate_sb[:], pg[:], AF.Exp, scale=-1.0)
        nc.vector.tensor_scalar(out=gate_sb[:], in0=gate_sb[:], scalar1=1.0,
                                scalar2=None, op0=OP.add)
        nc.vector.reciprocal(gate_sb[:], gate_sb[:])

        # ---- S[q, h, k] = qk logits + key mask (fused into the PSUM copy) ----
        s_sb = sing.tile([128, H, L], F32)
        for h in range(H):
            pS = ps.tile([128, L], F32, tag="p0")
            base = 32 * (h % 4)
            nc.tensor.matmul(pS[:], lhsT=qT[base:base + 32, h // 4, :],
                             rhs=kT[base:base + 32, h // 4, :],
                             start=True, stop=True, tile_position=(base, 0))
            nc.vector.tensor_tensor(out=s_sb[:, h, :], in0=pS[:],
                                    in1=mk_sb.rearrange("p a b -> p (a b)"), op=OP.add)

        # ---------------- phase 1: stream bias chunks ----------------
        p_all = sing.tile([128, H, L], BF)         # [q, h, k]

        # 3-stage software pipeline over 16 half-chunks:
        #   A(h): DVE square + PE projection matmuls
        #   B(h): PE sum-of-squares matmuls + ACT/DVE stats -> rinv
        #   C(h): DVE fixup (raw*rinv + S) + ACT exp -> p_all
        # emitted as C(h-2); A(h); B(h-1) so every engine's stream is
        # one stage ahead of its dependencies (no head-of-line blocking).
        tbs = []
        for ci in range(NCH):
            tb = ldp.tile([128, KC, D_BIAS], BF, tag="tb")
            nc.sync.dma_start(out=tb[:], in_=bias_tr[:, ci * KC:(ci + 1) * KC, :])
            tbs.append(tb)

        stA = {}
        stB = {}

        def stageA(h):
            ci, half = divmod(h, 2)
            tb = tbs[ci]
            sq = sqp.tile([128, HC, D_BIAS], BF, tag="sq")
            if h % 4 == 3:
                nc.scalar.activation(sq[:], tb[:, half * HC:(half + 1) * HC, :],
                                     AF.Square)
            else:
                nc.vector.tensor_tensor(out=sq[:], in0=tb[:, half * HC:(half + 1) * HC, :],
                                        in1=tb[:, half * HC:(half + 1) * HC, :], op=OP.mult)
            rp = ps_raw.tile([128, 512], F32, tag="rawps")
            for j in range(HC):
                nc.tensor.matmul(rp[:, j * 16:(j + 1) * 16],
                                 lhsT=tb[:, half * HC + j, :],
                                 rhs=wext_sb, start=True, stop=True)
            stA[h] = (sq, rp)

        def stageB(h):
            sq, rp = stA.pop(h)
            ss = ssps.tile([128, HC], F32, tag="sscol")
            for j in range(HC):
                nc.tensor.matmul(ss[:, j:j + 1], lhsT=sq[:, j, :],
                                 rhs=ones_col[:], start=True, stop=True)
            # rinv = exp(-0.5 * ln(var + eps)),  var = ss/128 - mean^2
            msq = scr.tile([128, HC], F32, tag="msq")
            nc.scalar.activation(msq[:], _ap(rp[:], 8, [[16, HC]]), AF.Square)
            var = scr.tile([128, HC], F32, tag="var")
            nc.vector.scalar_tensor_tensor(out=var[:], in0=ss[:],
                                           scalar=1.0 / D_BIAS, in1=msq[:],
                                           op0=OP.mult, op1=OP.subtract)
            lnv = scr.tile([128, HC], F32, tag="lnv")
            nc.scalar.activation(lnv[:], var[:], AF.Ln, bias=eps_sb[:, 0:1])
            rinv = scr.tile([128, HC], F32, tag="rinv")
            nc.scalar.activation(rinv[:], lnv[:], AF.Exp, scale=-0.5)
            stB[h] = (rp, rinv)

        def stageC(h):
            rp, rinv = stB.pop(h)
            k0 = h * HC
            t1 = scr.tile([128, H, HC], F32, tag="fx1")
            nc.vector.tensor_tensor(out=t1[:], in0=_ap(rp[:], 0, [[1, H], [16, HC]]),
                                    in1=_ap(rinv[:], 0, [[0, H], [1, HC]]), op=OP.mult)
            t2 = scr.tile([128, H, HC], F32, tag="fx2")
            nc.gpsimd.tensor_tensor(out=t2[:], in0=t1[:],
                                    in1=_ap(s_sb[:], k0, [[L, H], [1, HC]]), op=OP.add)
            nc.scalar.activation(_ap(p_all[:], k0, [[L, H], [1, HC]]), t2[:], AF.Exp)

        pta = sing.tile([128, H, 2, 128], BF)
        for h in range(2 * NCH + 2):
            if h >= 2:
                stageC(h - 2)
            if h == 11:
                # first half of every head's P^T: overlaps the rest of phase 1
                for hh in range(H):
                    nc.scalar.dma_start_transpose(pta[:, hh, :, :], p_all[:, hh, 0:256])
            if h < 2 * NCH:
                stageA(h)
            if h >= 1 and h - 1 < 2 * NCH:
                stageB(h - 1)

        # ---------------- phase 2: PV (+denominator as ones-column), output ----------------
        pvps_full = ps.tile([128, L], F32, tag="p0")
        pvps = pvps_full[:, 0:264]
        for h in range(H):
            ptb = pvp.tile([128, 2, 128], BF, tag="pt")
            eng = nc.sync if h % 2 == 0 else nc.scalar
            eng.dma_start_transpose(ptb[:], p_all[:, h, 256:512])
            for kc4 in range(4):
                pt_sl = pta[:, h, kc4, :] if kc4 < 2 else ptb[:, kc4 - 2, :]
                nc.tensor.matmul(pvps[:, h * (DH + 1):(h + 1) * (DH + 1)],
                                 lhsT=pt_sl,
                                 rhs=v_sb[:, kc4, h * (DH + 1):(h + 1) * (DH + 1)],
                                 start=(kc4 == 0), stop=(kc4 == 3))

        denr = sing.tile([128, H], F32)
        nc.vector.tensor_scalar(out=denr[:], in0=_ap(pvps[:], DH, [[DH + 1, H]]),
                                scalar1=1e-30, scalar2=None, op0=OP.add)
        nc.vector.reciprocal(denr[:], denr[:])

        comb = sing.tile([128, D_IN], BF)
        t = scr.tile([128, D_IN], F32, tag="comb_t")
        nc.vector.tensor_tensor(out=t[:].rearrange("p (h d) -> p h d", h=H),
                                in0=_ap(pvps[:], 0, [[DH + 1, H], [1, DH]]),
                                in1=gate_sb[:].rearrange("p (h d) -> p h d", h=H),
                                op=OP.mult)
        nc.vector.tensor_tensor(out=comb[:].rearrange("p (h d) -> p h d", h=H),
                                in0=t[:].rearrange("p (h d) -> p h d", h=H),
                                in1=_ap(denr[:], 0, [[1, H], [0, DH]]), op=OP.mult)

        fin_full = ps.tile([128, L], F32, tag="p0")
        fin = fin_full[:, 0:D_IN]
        cT = pvp.tile([128, 2, 128], BF, tag="cT")
        nc.sync.dma_start_transpose(cT[:], comb[:])
        for c in range(2):
            nc.tensor.matmul(fin[:], lhsT=cT[:, c, :], rhs=w_sb["o"][:, c, :],
                             start=(c == 0), stop=False)
        nc.tensor.matmul(fin[:], lhsT=ones_row[:, 0:128], rhs=brow_sb[:, 4, :],
                         start=False, stop=True)
        out_sb = sing.tile([128, D_IN], F32)
        nc.scalar.activation(out_sb[:], fin[:], AF.Copy, scale=rowm_sb[:, 0:1])
        nc.sync.dma_start(out=out[:, :], in_=out_sb[:])

    # Steer insert_act_table_loads to the one set that covers Ln/Exp/Copy
    # (otherwise it alternates exp_and_others <-> natural_log, ~19 table loads).
    # Hiding functions from other sets only restricts choices; ids stay intact.
    orig_tables = bacc.get_activation_tables
    keep = "natural_log_exp_and_others"

    def _patched(arch):
        t = orig_tables(arch)
        return {name: (fs if name == keep else set()) for name, fs in t.items()}

    bacc.get_activation_tables = _patched
    try:
        nc.compile()
    finally:
        bacc.get_activation_tables = orig_tables
    return nc


def _prep_common(inputs):
    ln_in_g = np.asarray(inputs["ln_in_g"], np.float64)
    ln_in_b = np.asarray(inputs["ln_in_b"], np.float64)
    ln_b_g = np.asarray(inputs["ln_b_g"], np.float64)
    Wq = np.asarray(inputs["Wq"], np.float64)
    Wk = np.asarray(inputs["Wk"], np.float64)
    Wv = np.asarray(inputs["Wv"], np.float64)
    Wg = np.asarray(inputs["Wg"], np.float64)
    Wb = np.asarray(inputs["Wb"], np.float64)
    Wo = np.asarray(inputs["Wo"], np.float64)
    bg = np.asarray(inputs["bg"], np.float64)
    bo = np.asarray(inputs["bo"], np.float64)

    def arr_w(w):  # [256, 256] -> [128, 2, 256] din-chunk grouping
        return np.ascontiguousarray(
            w.reshape(2, 128, D_IN).transpose(1, 0, 2)).astype(BF16)

    wall = np.zeros((128, 12, D_IN), BF16)
    wall[:, 0:2] = arr_w(Wq * ln_in_g[:, None])
    wall[:, 2:4] = arr_w(Wk * ln_in_g[:, None] * SCALE)
    wall[:, 4:6] = arr_w(Wv * ln_in_g[:, None])
    wall[:, 6:8] = arr_w(Wg * ln_in_g[:, None])
    wall[:, 8:10] = arr_w(Wo)

    brows = np.stack([
        ln_in_b @ Wq,
        (ln_in_b @ Wk) * SCALE,
        ln_in_b @ Wv,
        ln_in_b @ Wg + bg,
        bo,
    ]).astype(BF16)

    c1 = ln_b_g @ Wb                        # [H]
    wext = np.zeros((D_BIAS, 16), np.float64)
    # head cols pre-centered: T @ (g*Wb - c1/128) == T@ (g*Wb) - mean(T)*c1
    wext[:, 0:H] = Wb * ln_b_g[:, None] - c1[None, :] / D_BIAS
    wext[:, 8] = 1.0 / D_BIAS
    wall[:, 10, 0:16] = wext.astype(BF16)
    wall[:, 11, 0:128] = np.eye(128, dtype=BF16)

    return dict(wall=wall, brows=brows)


def _make_in_maps(inputs):
    x = np.asarray(inputs["x"], np.float32)
    bias = np.asarray(inputs["bias"], np.float32)
    mask = np.asarray(inputs["mask"])
    common = _prep_common(inputs)

    in_maps = []
    for c in range(8):
        b, qb = divmod(c, 4)
        q0 = qb * QB
        rowm = (mask[b, q0:q0 + QB] != 0).astype(np.float32)[:, None].copy()
        nat = bias[b, q0:q0 + QB].astype(BF16)
        x_all = np.empty((128, 7, D_IN), BF16)
        x_all[:, 0:4] = x[b].reshape(4, 128, D_IN).transpose(1, 0, 2)
        x_all[:, 4] = x[b, q0:q0 + QB]
        x_all[:, 5:7] = (np.broadcast_to(
            (mask[b] == 0).astype(np.float32) * NEG, (128, L))
            .reshape(128, 2, D_IN)).astype(BF16)
        xfull = np.concatenate([x[b], x[b, q0:q0 + QB]], axis=0)   # [640, 256]
        x_allT = np.ascontiguousarray(
            xfull.T.reshape(2, 128, 640).transpose(1, 0, 2)).astype(BF16)
        in_maps.append(dict(
            bias_tr=np.ascontiguousarray(nat.transpose(2, 1, 0)),
            x_all=x_all, x_allT=x_allT,
            rowm=rowm,
            **common,
        ))
    return in_maps


def kernel(**inputs):
    if "nc" not in _CACHE:
        _CACHE["nc"] = _build()
    nc = _CACHE["nc"]

    in_maps = _make_in_maps(inputs)
    res = run_bass_kernel_spmd(nc, in_maps, list(range(8)))
    out = np.empty((B, L, D_IN), np.float32)
    for c in range(8):
        b, qb = divmod(c, 4)
        out[b, qb * QB:(qb + 1) * QB] = res.results[c]["out"]
    return out


# revision 20
# speedup vs baseline: 1.2100x; 1.2100x over previous
"""AttentionWithBias (AlphaFold-style gated attention with pair bias) on 8 trn2 cores.

Sharding: core c handles batch b = c//4, query block qb = c%4 (128 queries).
Each core streams its [128, 512, 128] f32 bias slice from HBM ONCE, as a
host-side pre-transposed bf16 copy [d, k, q] (16 MB/core, ~47 us DMA) — half
the HBM traffic of the previous two-copy scheme.

Per key k the PE loads the [d=128, q=128] bias tile as stationary weights and
issues two matmuls:
  raw[q, 0:16] = tile^T @ wext      (cols 0..7 = g*Wb - c1/128, col 8 = 1/128)
  ss[q]        = sqtile^T @ ones    (sum of squares over d, out-free-size 1)
where sqtile = tile*tile is produced by one contiguous bf16 DVE multiply per
half-chunk (2x DVE mode).  This moves the LayerNorm variance reduction off
the vector/scalar engines (which were the bottleneck: a 67 us DVE add-tree +
35 us of ACT squares) onto the PE, where the reduction costs ~3 ns per key.

LayerNorm is folded into a post-matmul affine fixup as before:
  bias_h[r, h] = raw[r, h] * rinv_r   (+const_h, cancels in softmax)
  rinv = exp(-0.5*ln(var+eps)), var = ss/128 - mean^2, mean = raw[r, 8].

The key-side mask is folded into the S logits via a rank-1 PE matmul
(ones^T @ mask_row) accumulated into the same PSUM as q@k^T.  The softmax
denominator is folded into the PV matmuls as an extra ones-column (out col
256+h), so no separate DVE reduction is needed.  All fixup/exp tensors use a
[q, h, k] layout so every ACT write is innermost-contiguous.  Softmax runs
without max-subtraction (logits are O(10); masked lanes get -2e9 -> exp 0);
per-(q,h)-constant terms cancel in softmax so c2 and the query-side mask drop
out (fully-masked rows are zeroed by the final row mask).
"""

import sys

if "/opt/trn_rl_repo" not in sys.path:
    sys.path.insert(0, "/opt/trn_rl_repo")

from contextlib import ExitStack

import ml_dtypes
import numpy as np

import concourse.bacc as bacc
import concourse.bass as bass
import concourse.tile as tile
from concourse import mybir
from concourse.bass_utils import run_bass_kernel_spmd

BF16 = ml_dtypes.bfloat16
F32 = mybir.dt.float32
BF = mybir.dt.bfloat16
AF = mybir.ActivationFunctionType
OP = mybir.AluOpType

D_IN = 256
D_BIAS = 128
H = 8
DH = 32
B = 2
L = 512
SCALE = 1.0 / np.sqrt(DH)
QB = 128          # queries per core
KC = 64           # keys per streamed chunk
NCH = L // KC     # chunks
HC = 32           # keys per PSUM half
NEG = -2.0e9
EPS = 1e-5

_CACHE = {}


def _ap(base, off, dims):
    return bass.AP(tensor=base.tensor, offset=base.offset + off, ap=[list(base.ap[0])] + dims)


def _build():
    nc = bacc.Bacc("TRN2", target_bir_lowering=False, debug=False, num_devices=8)

    bias_tr = nc.declare_dram_parameter("bias_tr", [D_BIAS, L, QB], BF, isOutput=False)
    # x rows (4x128), x_q, key mask rows — one early DMA (stats + mask only)
    x_all = nc.declare_dram_parameter("x_all", [128, 5, D_IN], BF, isOutput=False)
    # host-transposed x: [din-chunk, 2, 512 batch rows + 128 q rows]
    x_allT = nc.declare_dram_parameter("x_allT", [128, 2, 640], BF, isOutput=False)
    # wq|wk|wv|wg|wo as [128, 2, 256] din-chunk groups; slot 10 = wext; slot 11 = I128
    wall = nc.declare_dram_parameter("wall", [128, 12, D_IN], BF, isOutput=False)
    rowm = nc.declare_dram_parameter("rowm", [128, 5], F32, isOutput=False)
    # per-projection row biases [1, 256] (ln_in_b folded through each W, + bg for gate)
    brows = nc.declare_dram_parameter("brows", [5, D_IN], BF, isOutput=False)

    out = nc.declare_dram_parameter("out", [QB, D_IN], F32, isOutput=True)

    with tile.TileContext(nc) as tc, ExitStack() as ctx:
        sing = ctx.enter_context(tc.tile_pool(name="sing", bufs=1))
        ldp = ctx.enter_context(tc.tile_pool(name="ldp", bufs=3))
        sqp = ctx.enter_context(tc.tile_pool(name="sqp", bufs=3))
        scr = ctx.enter_context(tc.tile_pool(name="scr", bufs=3))
        pvp = ctx.enter_context(tc.tile_pool(name="pvp", bufs=2))
        ps = ctx.enter_context(tc.tile_pool(name="ps", bufs=2, space="PSUM"))
        ps_raw = ctx.enter_context(tc.tile_pool(name="ps_raw", bufs=4, space="PSUM"))
        ssps = ctx.enter_context(tc.tile_pool(name="ssps", bufs=2, space="PSUM"))

        # ---------------- phase 0: batched early loads on SP ----------------
        xall_sb = sing.tile([128, 5, D_IN], BF)
        nc.sync.dma_start(out=xall_sb[:], in_=x_all[:, :, :])
        xallT_sb = sing.tile([128, 2, 640], BF)
        nc.sync.dma_start(out=xallT_sb[:], in_=x_allT[:, :, :])
        wall_sb = sing.tile([128, 12, D_IN], BF)
        nc.sync.dma_start(out=wall_sb[:], in_=wall[:, :, :])
        wext_sb = wall_sb[:, 10, 0:16]
        i128 = wall_sb[:, 11, 0:128]
        w_sb = {nm: wall_sb[:, 2 * i:2 * i + 2, :]
                for i, nm in enumerate(("q", "k", "v", "g", "o"))}
        m01 = None  # mask01 lives in rowm_sb[:, 1:5] (f32, for ACT scale)
        brow_sb = sing.tile([1, 5, D_IN], BF)
        nc.gpsimd.dma_start(out=brow_sb[:], in_=brows[None, :, :])
        ones_row = sing.tile([1, L], BF)
        nc.vector.memset(ones_row[:], 1.0)
        ones_col = sing.tile([128, 1], BF)
        nc.vector.memset(ones_col[:], 1.0)
        rowm_sb = sing.tile([128, 5], F32)
        nc.gpsimd.dma_start(out=rowm_sb[:], in_=rowm[:, :])
        eps_sb = sing.tile([128, 1], F32)
        nc.vector.memset(eps_sb[:], EPS)

        # ---- x LayerNorm, applied in transposed space ----
        # stats per row from the natural copy; then xnT = (xT - m) * rstd with
        # m/rstd broadcast along partitions via PE rank-1s (stat columns are
        # turned into rows by one PE transpose against I128).
        mv_all = sing.tile([128, 5, 2], F32)
        for r in range(5):
            st6 = scr.tile([128, 6], F32, tag="ln_st6")
            nc.vector.bn_stats(out=st6[:], in_=xall_sb[:, r, :])
            nc.vector.bn_aggr(out=mv_all[:, r, :], in_=st6[:])
        lnv5 = sing.tile([128, 5], F32)
        nc.scalar.activation(lnv5[:], _ap(mv_all[:], 1, [[2, 5]]), AF.Ln,
                             bias=eps_sb[:, 0:1])
        stat_bf = sing.tile([128, 2, 5], BF)
        nc.scalar.activation(stat_bf[:, 1, :], lnv5[:], AF.Exp, scale=-0.5)
        nc.vector.tensor_copy(stat_bf[:, 0, :], _ap(mv_all[:], 0, [[2, 5]]))
        statT = sing.tile([1, 1280], BF)
        for g in range(3):
            pstT = ps.tile([128, L], F32, tag="p0")
            n = min(4, 10 - g * 4)
            for j in range(n):
                stat, blk = divmod(g * 4 + j, 5)
                nc.tensor.matmul(pstT[0:1, j * 128:(j + 1) * 128],
                                 lhsT=stat_bf[:, stat, blk:blk + 1],
                                 rhs=i128, start=True, stop=True)
            nc.scalar.copy(statT[:, g * 512:g * 512 + n * 128],
                           pstT[0:1, 0:n * 128])

        xnT = sing.tile([128, 2, 640], BF)
        for blk in range(5):
            pbr = ps.tile([128, L], F32, tag="p0")
            nc.tensor.matmul(pbr[:, 0:128], lhsT=ones_row[:, 0:QB],
                             rhs=statT[:, blk * 128:(blk + 1) * 128],
                             start=True, stop=True)
            nc.tensor.matmul(pbr[:, 128:256], lhsT=ones_row[:, 0:QB],
                             rhs=statT[:, (5 + blk) * 128:(6 + blk) * 128],
                             start=True, stop=True)
            csl = slice(blk * 128, (blk + 1) * 128)
            tx = scr.tile([128, 2, 128], F32, tag="xnt_t")
            nc.vector.tensor_tensor(out=tx[:], in0=xallT_sb[:, :, csl],
                                    in1=_ap(pbr[:], 0, [[0, 2], [1, 128]]),
                                    op=OP.subtract)
            nc.vector.tensor_tensor(out=xnT[:, :, csl], in0=tx[:],
                                    in1=_ap(pbr[:], 128, [[0, 2], [1, 128]]),
                                    op=OP.mult)

        # ---- kT, qT ----
        kT = sing.tile([128, 2, L], BF)
        for h2 in range(2):
            pk = ps.tile([128, L], F32, tag="p0")
            nc.tensor.matmul(pk[:], lhsT=w_sb["k"][:, 0, h2 * 128:(h2 + 1) * 128],
                             rhs=xnT[:, 0, 0:L], start=True, stop=False)
            nc.tensor.matmul(pk[:], lhsT=w_sb["k"][:, 1, h2 * 128:(h2 + 1) * 128],
                             rhs=xnT[:, 1, 0:L], start=False, stop=False)
            nc.tensor.matmul(pk[:], lhsT=brow_sb[:, 1, h2 * 128:(h2 + 1) * 128],
                             rhs=ones_row[:], start=False, stop=True)
            nc.scalar.copy(kT[:, h2, :], pk[:])
        qT = sing.tile([128, 2, QB], BF)
        for h2 in range(2):
            pq = ps.tile([128, QB], F32, tag="p0")
            nc.tensor.matmul(pq[:], lhsT=w_sb["q"][:, 0, h2 * 128:(h2 + 1) * 128],
                             rhs=xnT[:, 0, 512:640], start=True, stop=False)
            nc.tensor.matmul(pq[:], lhsT=w_sb["q"][:, 1, h2 * 128:(h2 + 1) * 128],
                             rhs=xnT[:, 1, 512:640], start=False, stop=False)
            nc.tensor.matmul(pq[:], lhsT=brow_sb[:, 0, h2 * 128:(h2 + 1) * 128],
                             rhs=ones_row[:, 0:QB], start=False, stop=True)
            nc.scalar.copy(qT[:, h2, :], pq[:])

        # ---- v_ext (natural [k rows, h*(dh+1)]) with a per-head ones column
        #      so the PV matmul also accumulates the softmax denominator ----
        v_sb = sing.tile([128, 4, H * (DH + 1)], BF)
        for r in range(4):
            pv = ps.tile([128, D_IN], F32, tag="p0")
            nc.tensor.matmul(pv[:], lhsT=xnT[:, 0, r * 128:(r + 1) * 128],
                             rhs=w_sb["v"][:, 0, :], start=True, stop=False)
            nc.tensor.matmul(pv[:], lhsT=xnT[:, 1, r * 128:(r + 1) * 128],
                             rhs=w_sb["v"][:, 1, :], start=False, stop=False)
            nc.tensor.matmul(pv[:], lhsT=ones_row[:, 0:128],
                             rhs=brow_sb[:, 2, :], start=False, stop=True)
            nc.scalar.activation(_ap(v_sb[:, r, :], 0, [[DH + 1, H], [1, DH]]),
                                 pv[:].rearrange("p (h d) -> p h d", h=H),
                                 AF.Copy, scale=rowm_sb[:, r + 1:r + 2])
            nc.scalar.copy(_ap(v_sb[:, r, :], DH, [[DH + 1, H]]),
                           _ap(rowm_sb[:, r + 1:r + 2], 0, [[0, H]]))

        # ---- gate = sigmoid(xq @ Wg + bgate) ----
        gate_sb = sing.tile([128, D_IN], F32)
        pg = ps.tile([128, D_IN], F32, tag="p0")
        nc.tensor.matmul(pg[:], lhsT=xnT[:, 0, 512:640], rhs=w_sb["g"][:, 0, :],
                         start=True, stop=False)
        nc.tensor.matmul(pg[:], lhsT=xnT[:, 1, 512:640], rhs=w_sb["g"][:, 1, :],
                         start=False, stop=False)
        nc.tensor.matmul(pg[:], lhsT=ones_row[:, 0:128], rhs=brow_sb[:, 3, :],
                         start=False, stop=True)
        # sigmoid(x) = 1/(1+exp(-x)) — avoids loading the sigmoid ACT table set
        nc.scalar.activation(gate_sb[:], pg[:], AF.Exp, scale=-1.0)
        nc.vector.tensor_scalar(out=gate_sb[:], in0=gate_sb[:], scalar1=1.0,
                                scalar2=None, op0=OP.add)
        nc.vector.reciprocal(gate_sb[:], gate_sb[:])

        # ---- S[q, h, k] = qk logits (masking is handled via zeroed V rows
        #      and the mask01 denominator column — exact softmax exclusion) ----
        s_sb = sing.tile([128, H, L], F32)
        for h in range(H):
            pS = ps.tile([128, L], F32, tag="p0")
            base = 32 * (h % 4)
            nc.tensor.matmul(pS[:], lhsT=qT[base:base + 32, h // 4, :],
                             rhs=kT[base:base + 32, h // 4, :],
                             start=True, stop=True, tile_position=(base, 0))
            nc.scalar.copy(s_sb[:, h, :], pS[:])

        # ---------------- phase 1: stream bias chunks ----------------
        p_all = sing.tile([128, H, L], BF)         # [q, h, k]

        # 3-stage software pipeline over 16 half-chunks:
        #   A(h): DVE square + PE projection matmuls
        #   B(h): PE sum-of-squares matmuls + ACT/DVE stats -> rinv
        #   C(h): DVE fixup (raw*rinv + S) + ACT exp -> p_all
        # emitted as C(h-2); A(h); B(h-1) so every engine's stream is
        # one stage ahead of its dependencies (no head-of-line blocking).
        tbs = []
        for ci in range(NCH):
            tb = ldp.tile([128, KC, D_BIAS], BF, tag="tb")
            nc.sync.dma_start(out=tb[:], in_=bias_tr[:, ci * KC:(ci + 1) * KC, :])
            tbs.append(tb)

        stA = {}
        stB = {}

        def stageA(h):
            ci, half = divmod(h, 2)
            tb = tbs[ci]
            sq = sqp.tile([128, HC, D_BIAS], BF, tag="sq")
            nc.vector.tensor_tensor(out=sq[:], in0=tb[:, half * HC:(half + 1) * HC, :],
                                    in1=tb[:, half * HC:(half + 1) * HC, :], op=OP.mult)
            rp = ps_raw.tile([128, 512], F32, tag="rawps")
            for j in range(HC):
                nc.tensor.matmul(rp[:, j * 16:(j + 1) * 16],
                                 lhsT=tb[:, half * HC + j, :],
                                 rhs=wext_sb, start=True, stop=True)
            stA[h] = (sq, rp)

        def stageB(h):
            sq, rp = stA.pop(h)
            ss = ssps.tile([128, HC], F32, tag="sscol")
            for j in range(HC):
                nc.tensor.matmul(ss[:, j:j + 1], lhsT=sq[:, j, :],
                                 rhs=ones_col[:], start=True, stop=True)
            # rinv = exp(-0.5 * ln(var + eps)),  var = ss/128 - mean^2
            msq = scr.tile([128, HC], F32, tag="msq")
            nc.scalar.activation(msq[:], _ap(rp[:], 8, [[16, HC]]), AF.Square)
            var = scr.tile([128, HC], F32, tag="var")
            nc.vector.scalar_tensor_tensor(out=var[:], in0=ss[:],
                                           scalar=1.0 / D_BIAS, in1=msq[:],
                                           op0=OP.mult, op1=OP.subtract)
            lnv = scr.tile([128, HC], F32, tag="lnv")
            nc.scalar.activation(lnv[:], var[:], AF.Ln, bias=eps_sb[:, 0:1])
            rinv = scr.tile([128, HC], F32, tag="rinv")
            nc.scalar.activation(rinv[:], lnv[:], AF.Exp, scale=-0.5)
            stB[h] = (rp, rinv)

        def stageC(h):
            rp, rinv = stB.pop(h)
            k0 = h * HC
            t1 = scr.tile([128, H, HC], F32, tag="fx1")
            nc.vector.tensor_tensor(out=t1[:], in0=_ap(rp[:], 0, [[1, H], [16, HC]]),
                                    in1=_ap(rinv[:], 0, [[0, H], [1, HC]]), op=OP.mult)
            t2 = scr.tile([128, H, HC], F32, tag="fx2")
            nc.vector.tensor_tensor(out=t2[:], in0=t1[:],
                                    in1=_ap(s_sb[:], k0, [[L, H], [1, HC]]), op=OP.add)
            nc.scalar.activation(_ap(p_all[:], k0, [[L, H], [1, HC]]), t2[:], AF.Exp)

        pta = sing.tile([128, H, 2, 128], BF)
        for h in range(2 * NCH + 2):
            if h >= 2:
                stageC(h - 2)
            if h == 11:
                # first half of every head's P^T: overlaps the rest of phase 1
                for hh in range(H):
                    nc.scalar.dma_start_transpose(pta[:, hh, :, :], p_all[:, hh, 0:256])
            if h < 2 * NCH:
                stageA(h)
            if h >= 1 and h - 1 < 2 * NCH:
                stageB(h - 1)

        # ---------------- phase 2: PV (+denominator as ones-column), output ----------------
        pvps_full = ps.tile([128, L], F32, tag="p0")
        pvps = pvps_full[:, 0:264]
        for h in range(H):
            ptb = pvp.tile([128, 2, 128], BF, tag="pt")
            eng = nc.sync if h % 2 == 0 else nc.scalar
            eng.dma_start_transpose(ptb[:], p_all[:, h, 256:512])
            for kc4 in range(4):
                pt_sl = pta[:, h, kc4, :] if kc4 < 2 else ptb[:, kc4 - 2, :]
                nc.tensor.matmul(pvps[:, h * (DH + 1):(h + 1) * (DH + 1)],
                                 lhsT=pt_sl,
                                 rhs=v_sb[:, kc4, h * (DH + 1):(h + 1) * (DH + 1)],
                                 start=(kc4 == 0), stop=(kc4 == 3))

        denr = sing.tile([128, H], F32)
        nc.vector.tensor_scalar(out=denr[:], in0=_ap(pvps[:], DH, [[DH + 1, H]]),
                                scalar1=1e-30, scalar2=None, op0=OP.add)
        nc.vector.reciprocal(denr[:], denr[:])

        comb = sing.tile([128, D_IN], BF)
        t = scr.tile([128, D_IN], F32, tag="comb_t")
        nc.vector.tensor_tensor(out=t[:].rearrange("p (h d) -> p h d", h=H),
                                in0=_ap(pvps[:], 0, [[DH + 1, H], [1, DH]]),
                                in1=gate_sb[:].rearrange("p (h d) -> p h d", h=H),
                                op=OP.mult)
        nc.vector.tensor_tensor(out=comb[:].rearrange("p (h d) -> p h d", h=H),
                                in0=t[:].rearrange("p (h d) -> p h d", h=H),
                                in1=_ap(denr[:], 0, [[1, H], [0, DH]]), op=OP.mult)

        fin_full = ps.tile([128, L], F32, tag="p0")
        fin = fin_full[:, 0:D_IN]
        cT = pvp.tile([128, 2, 128], BF, tag="cT")
        nc.sync.dma_start_transpose(cT[:], comb[:])
        for c in range(2):
            nc.tensor.matmul(fin[:], lhsT=cT[:, c, :], rhs=w_sb["o"][:, c, :],
                             start=(c == 0), stop=False)
        nc.tensor.matmul(fin[:], lhsT=ones_row[:, 0:128], rhs=brow_sb[:, 4, :],
                         start=False, stop=True)
        out_sb = sing.tile([128, D_IN], F32)
        nc.scalar.activation(out_sb[:], fin[:], AF.Copy, scale=rowm_sb[:, 0:1])
        nc.sync.dma_start(out=out[:, :], in_=out_sb[:])

    # Steer insert_act_table_loads to the one set that covers Ln/Exp/Copy
    # (otherwise it alternates exp_and_others <-> natural_log, ~19 table loads).
    # Hiding functions from other sets only restricts choices; ids stay intact.
    orig_tables = bacc.get_activation_tables
    keep = "natural_log_exp_and_others"

    def _patched(arch):
        t = orig_tables(arch)
        return {name: (fs if name == keep else set()) for name, fs in t.items()}

    bacc.get_activation_tables = _patched
    try:
        nc.compile()
    finally:
        bacc.get_activation_tables = orig_tables
    return nc


def _prep_common(inputs):
    ln_in_g = np.asarray(inputs["ln_in_g"], np.float64)
    ln_in_b = np.asarray(inputs["ln_in_b"], np.float64)
    ln_b_g = np.asarray(inputs["ln_b_g"], np.float64)
    Wq = np.asarray(inputs["Wq"], np.float64)
    Wk = np.asarray(inputs["Wk"], np.float64)
    Wv = np.asarray(inputs["Wv"], np.float64)
    Wg = np.asarray(inputs["Wg"], np.float64)
    Wb = np.asarray(inputs["Wb"], np.float64)
    Wo = np.asarray(inputs["Wo"], np.float64)
    bg = np.asarray(inputs["bg"], np.float64)
    bo = np.asarray(inputs["bo"], np.float64)

    def arr_w(w):  # [256, 256] -> [128, 2, 256] din-chunk grouping
        return np.ascontiguousarray(
            w.reshape(2, 128, D_IN).transpose(1, 0, 2)).astype(BF16)

    wall = np.zeros((128, 12, D_IN), BF16)
    wall[:, 0:2] = arr_w(Wq * ln_in_g[:, None])
    wall[:, 2:4] = arr_w(Wk * ln_in_g[:, None] * SCALE)
    wall[:, 4:6] = arr_w(Wv * ln_in_g[:, None])
    wall[:, 6:8] = arr_w(Wg * ln_in_g[:, None])
    wall[:, 8:10] = arr_w(Wo)

    brows = np.stack([
        ln_in_b @ Wq,
        (ln_in_b @ Wk) * SCALE,
        ln_in_b @ Wv,
        ln_in_b @ Wg + bg,
        bo,
    ]).astype(BF16)

    c1 = ln_b_g @ Wb                        # [H]
    wext = np.zeros((D_BIAS, 16), np.float64)
    # head cols pre-centered: T @ (g*Wb - c1/128) == T@ (g*Wb) - mean(T)*c1
    wext[:, 0:H] = Wb * ln_b_g[:, None] - c1[None, :] / D_BIAS
    wext[:, 8] = 1.0 / D_BIAS
    wall[:, 10, 0:16] = wext.astype(BF16)
    wall[:, 11, 0:128] = np.eye(128, dtype=BF16)

    return dict(wall=wall, brows=brows)


def _make_in_maps(inputs):
    x = np.asarray(inputs["x"], np.float32)
    bias = np.asarray(inputs["bias"], np.float32)
    mask = np.asarray(inputs["mask"])
    common = _prep_common(inputs)

    in_maps = []
    for c in range(8):
        b, qb = divmod(c, 4)
        q0 = qb * QB
        rowm = np.zeros((128, 5), np.float32)
        rowm[:, 0] = (mask[b, q0:q0 + QB] != 0)
        rowm[:, 1:5] = (mask[b] != 0).astype(np.float32).reshape(4, 128).T
        nat = bias[b, q0:q0 + QB].astype(BF16)
        x_all = np.zeros((128, 5, D_IN), BF16)
        x_all[:, 0:4] = x[b].reshape(4, 128, D_IN).transpose(1, 0, 2)
        x_all[:, 4] = x[b, q0:q0 + QB]
        xfull = np.concatenate([x[b], x[b, q0:q0 + QB]], axis=0)   # [640, 256]
        x_allT = np.ascontiguousarray(
            xfull.T.reshape(2, 128, 640).transpose(1, 0, 2)).astype(BF16)
        in_maps.append(dict(
            bias_tr=np.ascontiguousarray(nat.transpose(2, 1, 0)),
            x_all=x_all, x_allT=x_allT,
            rowm=rowm,
            **common,
        ))
    return in_maps


def kernel(**inputs):
    if "nc" not in _CACHE:
        _CACHE["nc"] = _build()
    nc = _CACHE["nc"]

    in_maps = _make_in_maps(inputs)
    res = run_bass_kernel_spmd(nc, in_maps, list(range(8)))
    out = np.empty((B, L, D_IN), np.float32)
    for c in range(8):
        b, qb = divmod(c, 4)
        out[b, qb * QB:(qb + 1) * QB] = res.results[c]["out"]
    return out


# revision 21
# speedup vs baseline: 1.3921x; 1.1505x over previous
"""AttentionWithBias (AlphaFold-style gated attention with pair bias) on 8 trn2 cores.

Sharding: core c handles batch b = c//4, query block qb = c%4 (128 queries).
Each core streams its [128, 512, 128] f32 bias slice from HBM ONCE, as a
host-side pre-transposed bf16 copy [d, k, q] (16 MB/core, ~47 us DMA) — half
the HBM traffic of the previous two-copy scheme.

Per key k the PE loads the [d=128, q=128] bias tile as stationary weights and
issues two matmuls:
  raw[q, 0:16] = tile^T @ wext      (cols 0..7 = g*Wb - c1/128, col 8 = 1/128)
  ss[q]        = sqtile^T @ ones    (sum of squares over d, out-free-size 1)
where sqtile = tile*tile is produced by one contiguous bf16 DVE multiply per
half-chunk (2x DVE mode).  This moves the LayerNorm variance reduction off
the vector/scalar engines (which were the bottleneck: a 67 us DVE add-tree +
35 us of ACT squares) onto the PE, where the reduction costs ~3 ns per key.

LayerNorm is folded into a post-matmul affine fixup as before:
  bias_h[r, h] = raw[r, h] * rinv_r   (+const_h, cancels in softmax)
  rinv = exp(-0.5*ln(var+eps)), var = ss/128 - mean^2, mean = raw[r, 8].

The key-side mask is folded into the S logits via a rank-1 PE matmul
(ones^T @ mask_row) accumulated into the same PSUM as q@k^T.  The softmax
denominator is folded into the PV matmuls as an extra ones-column (out col
256+h), so no separate DVE reduction is needed.  All fixup/exp tensors use a
[q, h, k] layout so every ACT write is innermost-contiguous.  Softmax runs
without max-subtraction (logits are O(10); masked lanes get -2e9 -> exp 0);
per-(q,h)-constant terms cancel in softmax so c2 and the query-side mask drop
out (fully-masked rows are zeroed by the final row mask).
"""

import sys

if "/opt/trn_rl_repo" not in sys.path:
    sys.path.insert(0, "/opt/trn_rl_repo")

from contextlib import ExitStack

import ml_dtypes
import numpy as np

import concourse.bacc as bacc
import concourse.bass as bass
import concourse.tile as tile
from concourse import mybir
from concourse.bass_utils import run_bass_kernel_spmd

BF16 = ml_dtypes.bfloat16
F32 = mybir.dt.float32
BF = mybir.dt.bfloat16
AF = mybir.ActivationFunctionType
OP = mybir.AluOpType

D_IN = 256
D_BIAS = 128
H = 8
DH = 32
B = 2
L = 512
SCALE = 1.0 / np.sqrt(DH)
QB = 128          # queries per core
KC = 64           # keys per streamed chunk
NCH = L // KC     # chunks
HC = 32           # keys per PSUM half
NEG = -2.0e9
EPS = 1e-5

_CACHE = {}


def _ap(base, off, dims):
    return bass.AP(tensor=base.tensor, offset=base.offset + off, ap=[list(base.ap[0])] + dims)


def _build():
    nc = bacc.Bacc("TRN2", target_bir_lowering=False, debug=False, num_devices=8)

    bias_tr = nc.declare_dram_parameter("bias_tr", [D_BIAS, L, QB], BF, isOutput=False)
    # x rows (4x128), x_q, key mask rows — one early DMA (stats + mask only)
    x_all = nc.declare_dram_parameter("x_all", [128, 5, D_IN], BF, isOutput=False)
    # host-transposed x: [din-chunk, 2, 512 batch rows + 128 q rows]
    x_allT = nc.declare_dram_parameter("x_allT", [128, 2, 640], BF, isOutput=False)
    # wq|wk|wv|wg|wo as [128, 2, 256] din-chunk groups; slot 10 = wext; slot 11 = I128
    wall = nc.declare_dram_parameter("wall", [128, 12, D_IN], BF, isOutput=False)
    rowm = nc.declare_dram_parameter("rowm", [128, 5], F32, isOutput=False)
    # per-projection row biases [1, 256] (ln_in_b folded through each W, + bg for gate)
    brows = nc.declare_dram_parameter("brows", [5, D_IN], BF, isOutput=False)

    out = nc.declare_dram_parameter("out", [QB, D_IN], F32, isOutput=True)

    with tile.TileContext(nc) as tc, ExitStack() as ctx:
        sing = ctx.enter_context(tc.tile_pool(name="sing", bufs=1))
        ldp = ctx.enter_context(tc.tile_pool(name="ldp", bufs=3))
        sqp = ctx.enter_context(tc.tile_pool(name="sqp", bufs=3))
        scr = ctx.enter_context(tc.tile_pool(name="scr", bufs=3))
        pvp = ctx.enter_context(tc.tile_pool(name="pvp", bufs=2))
        ps = ctx.enter_context(tc.tile_pool(name="ps", bufs=2, space="PSUM"))
        ps_raw = ctx.enter_context(tc.tile_pool(name="ps_raw", bufs=4, space="PSUM"))
        ssps = ctx.enter_context(tc.tile_pool(name="ssps", bufs=2, space="PSUM"))

        # ---------------- phase 0: batched early loads on SP ----------------
        xall_sb = sing.tile([128, 5, D_IN], BF)
        nc.sync.dma_start(out=xall_sb[:], in_=x_all[:, :, :])
        xallT_sb = sing.tile([128, 2, 640], BF)
        nc.sync.dma_start(out=xallT_sb[:], in_=x_allT[:, :, :])
        wall_sb = sing.tile([128, 12, D_IN], BF)
        nc.sync.dma_start(out=wall_sb[:], in_=wall[:, :, :])
        wext_sb = wall_sb[:, 10, 0:16]
        i128 = wall_sb[:, 11, 0:128]
        w_sb = {nm: wall_sb[:, 2 * i:2 * i + 2, :]
                for i, nm in enumerate(("q", "k", "v", "g", "o"))}
        m01 = None  # mask01 lives in rowm_sb[:, 1:5] (f32, for ACT scale)
        brow_sb = sing.tile([1, 5, D_IN], BF)
        nc.gpsimd.dma_start(out=brow_sb[:], in_=brows[None, :, :])
        ones_row = sing.tile([1, L], BF)
        nc.vector.memset(ones_row[:], 1.0)
        ones_col = sing.tile([128, 1], BF)
        nc.vector.memset(ones_col[:], 1.0)
        rowm_sb = sing.tile([128, 5], F32)
        nc.gpsimd.dma_start(out=rowm_sb[:], in_=rowm[:, :])
        eps_sb = sing.tile([128, 1], F32)
        nc.vector.memset(eps_sb[:], EPS)

        # ---- x LayerNorm, applied in transposed space ----
        # stats per row from the natural copy; then xnT = (xT - m) * rstd with
        # m/rstd broadcast along partitions via PE rank-1s (stat columns are
        # turned into rows by one PE transpose against I128).
        mv_all = sing.tile([128, 5, 2], F32)
        for r in range(5):
            st6 = scr.tile([128, 6], F32, tag="ln_st6")
            nc.vector.bn_stats(out=st6[:], in_=xall_sb[:, r, :])
            nc.vector.bn_aggr(out=mv_all[:, r, :], in_=st6[:])
        lnv5 = sing.tile([128, 5], F32)
        nc.scalar.activation(lnv5[:], _ap(mv_all[:], 1, [[2, 5]]), AF.Ln,
                             bias=eps_sb[:, 0:1])
        stat_bf = sing.tile([128, 2, 5], BF)
        nc.scalar.activation(stat_bf[:, 1, :], lnv5[:], AF.Exp, scale=-0.5)
        nc.vector.tensor_copy(stat_bf[:, 0, :], _ap(mv_all[:], 0, [[2, 5]]))
        statT = sing.tile([1, 1280], BF)
        for g in range(3):
            pstT = ps.tile([128, L], F32, tag="p0")
            n = min(4, 10 - g * 4)
            for j in range(n):
                stat, blk = divmod(g * 4 + j, 5)
                nc.tensor.matmul(pstT[0:1, j * 128:(j + 1) * 128],
                                 lhsT=stat_bf[:, stat, blk:blk + 1],
                                 rhs=i128, start=True, stop=True)
            nc.scalar.copy(statT[:, g * 512:g * 512 + n * 128],
                           pstT[0:1, 0:n * 128])

        xnT = sing.tile([128, 2, 640], BF)
        for blk in range(5):
            pbr = ps.tile([128, L], F32, tag="p0")
            nc.tensor.matmul(pbr[:, 0:128], lhsT=ones_row[:, 0:QB],
                             rhs=statT[:, blk * 128:(blk + 1) * 128],
                             start=True, stop=True)
            nc.tensor.matmul(pbr[:, 128:256], lhsT=ones_row[:, 0:QB],
                             rhs=statT[:, (5 + blk) * 128:(6 + blk) * 128],
                             start=True, stop=True)
            csl = slice(blk * 128, (blk + 1) * 128)
            tx = scr.tile([128, 2, 128], F32, tag="xnt_t")
            nc.vector.tensor_tensor(out=tx[:], in0=xallT_sb[:, :, csl],
                                    in1=_ap(pbr[:], 0, [[0, 2], [1, 128]]),
                                    op=OP.subtract)
            nc.vector.tensor_tensor(out=xnT[:, :, csl], in0=tx[:],
                                    in1=_ap(pbr[:], 128, [[0, 2], [1, 128]]),
                                    op=OP.mult)

        # ---- kT, qT ----
        kT = sing.tile([128, 2, L], BF)
        for h2 in range(2):
            pk = ps.tile([128, L], F32, tag="p0")
            nc.tensor.matmul(pk[:], lhsT=w_sb["k"][:, 0, h2 * 128:(h2 + 1) * 128],
                             rhs=xnT[:, 0, 0:L], start=True, stop=False)
            nc.tensor.matmul(pk[:], lhsT=w_sb["k"][:, 1, h2 * 128:(h2 + 1) * 128],
                             rhs=xnT[:, 1, 0:L], start=False, stop=False)
            nc.tensor.matmul(pk[:], lhsT=brow_sb[:, 1, h2 * 128:(h2 + 1) * 128],
                             rhs=ones_row[:], start=False, stop=True)
            nc.scalar.copy(kT[:, h2, :], pk[:])
        qT = sing.tile([128, 2, QB], BF)
        for h2 in range(2):
            pq = ps.tile([128, QB], F32, tag="p0")
            nc.tensor.matmul(pq[:], lhsT=w_sb["q"][:, 0, h2 * 128:(h2 + 1) * 128],
                             rhs=xnT[:, 0, 512:640], start=True, stop=False)
            nc.tensor.matmul(pq[:], lhsT=w_sb["q"][:, 1, h2 * 128:(h2 + 1) * 128],
                             rhs=xnT[:, 1, 512:640], start=False, stop=False)
            nc.tensor.matmul(pq[:], lhsT=brow_sb[:, 0, h2 * 128:(h2 + 1) * 128],
                             rhs=ones_row[:, 0:QB], start=False, stop=True)
            nc.scalar.copy(qT[:, h2, :], pq[:])

        # ---- v_ext (natural [k rows, h*(dh+1)]) with a per-head ones column
        #      so the PV matmul also accumulates the softmax denominator ----
        v_sb = sing.tile([128, 4, H * (DH + 1)], BF)
        for r in range(4):
            pv = ps.tile([128, D_IN], F32, tag="p0")
            nc.tensor.matmul(pv[:], lhsT=xnT[:, 0, r * 128:(r + 1) * 128],
                             rhs=w_sb["v"][:, 0, :], start=True, stop=False)
            nc.tensor.matmul(pv[:], lhsT=xnT[:, 1, r * 128:(r + 1) * 128],
                             rhs=w_sb["v"][:, 1, :], start=False, stop=False)
            nc.tensor.matmul(pv[:], lhsT=ones_row[:, 0:128],
                             rhs=brow_sb[:, 2, :], start=False, stop=True)
            nc.scalar.activation(_ap(v_sb[:, r, :], 0, [[DH + 1, H], [1, DH]]),
                                 pv[:].rearrange("p (h d) -> p h d", h=H),
                                 AF.Copy, scale=rowm_sb[:, r + 1:r + 2])
            nc.scalar.copy(_ap(v_sb[:, r, :], DH, [[DH + 1, H]]),
                           _ap(rowm_sb[:, r + 1:r + 2], 0, [[0, H]]))

        # ---- gate = sigmoid(xq @ Wg + bgate) ----
        gate_sb = sing.tile([128, D_IN], F32)
        pg = ps.tile([128, D_IN], F32, tag="p0")
        nc.tensor.matmul(pg[:], lhsT=xnT[:, 0, 512:640], rhs=w_sb["g"][:, 0, :],
                         start=True, stop=False)
        nc.tensor.matmul(pg[:], lhsT=xnT[:, 1, 512:640], rhs=w_sb["g"][:, 1, :],
                         start=False, stop=False)
        nc.tensor.matmul(pg[:], lhsT=ones_row[:, 0:128], rhs=brow_sb[:, 3, :],
                         start=False, stop=True)
        # sigmoid(x) = 1/(1+exp(-x)) — avoids loading the sigmoid ACT table set
        nc.scalar.activation(gate_sb[:], pg[:], AF.Exp, scale=-1.0)
        nc.vector.tensor_scalar(out=gate_sb[:], in0=gate_sb[:], scalar1=1.0,
                                scalar2=None, op0=OP.add)
        nc.vector.reciprocal(gate_sb[:], gate_sb[:])

        # ---- S[q, h, k] = qk logits (masking is handled via zeroed V rows
        #      and the mask01 denominator column — exact softmax exclusion) ----
        s_sb = sing.tile([128, H, L], F32)
        for h in range(H):
            pS = ps.tile([128, L], F32, tag="p0")
            base = 32 * (h % 4)
            nc.tensor.matmul(pS[:], lhsT=qT[base:base + 32, h // 4, :],
                             rhs=kT[base:base + 32, h // 4, :],
                             start=True, stop=True, tile_position=(base, 0))
            nc.scalar.copy(s_sb[:, h, :], pS[:])

        # ---------------- phase 1: stream bias chunks ----------------
        p_all = sing.tile([128, H, L], BF)         # [q, h, k]

        # 3-stage software pipeline over 16 half-chunks:
        #   A(h): DVE square + PE projection matmuls
        #   B(h): PE sum-of-squares matmuls + ACT/DVE stats -> rinv
        #   C(h): DVE fixup (raw*rinv + S) + ACT exp -> p_all
        # emitted as C(h-2); A(h); B(h-1) so every engine's stream is
        # one stage ahead of its dependencies (no head-of-line blocking).
        tbs = []
        for ci in range(NCH):
            tb = ldp.tile([128, KC, D_BIAS], BF, tag="tb")
            nc.sync.dma_start(out=tb[:], in_=bias_tr[:, ci * KC:(ci + 1) * KC, :])
            tbs.append(tb)

        stA = {}
        stB = {}

        def stageA(h):
            ci, half = divmod(h, 2)
            tb = tbs[ci]
            sq = sqp.tile([128, HC, D_BIAS], BF, tag="sq")
            nc.vector.tensor_tensor(out=sq[:], in0=tb[:, half * HC:(half + 1) * HC, :],
                                    in1=tb[:, half * HC:(half + 1) * HC, :], op=OP.mult)
            rp = ps_raw.tile([128, 512], F32, tag="rawps")
            for j in range(HC):
                nc.tensor.matmul(rp[:, j * 16:(j + 1) * 16],
                                 lhsT=tb[:, half * HC + j, :],
                                 rhs=wext_sb, start=True, stop=True)
            stA[h] = (sq, rp)

        def stageB(h):
            # sum-of-squares columns + per-half mean^2; chunk tiles fill by half
            ci, half = divmod(h, 2)
            if half == 0:
                ss = ssps.tile([128, KC], F32, tag="sscol")
                msqc = scr.tile([128, KC], F32, tag="msqc")
                stB[ci] = [ss, msqc, stA[h][1], None]
            else:
                ss, msqc = stB[ci][0], stB[ci][1]
                stB[ci][3] = stA[h][1]
            sq, rp = stA.pop(h)
            for j in range(HC):
                nc.tensor.matmul(ss[:, half * HC + j:half * HC + j + 1],
                                 lhsT=sq[:, j, :],
                                 rhs=ones_col[:], start=True, stop=True)
            nc.scalar.activation(msqc[:, half * HC:(half + 1) * HC],
                                 _ap(rp[:], 8, [[16, HC]]), AF.Square)

        def stageC(ci):
            # per-chunk: var -> rinv, fixup t1 (per half, PSUM) + t2 + exp
            ss, msqc, rp0, rp1 = stB.pop(ci)
            var = scr.tile([128, KC], F32, tag="var")
            nc.vector.scalar_tensor_tensor(out=var[:], in0=ss[:],
                                           scalar=1.0 / D_BIAS, in1=msqc[:],
                                           op0=OP.mult, op1=OP.subtract)
            lnv = scr.tile([128, KC], F32, tag="lnv")
            nc.scalar.activation(lnv[:], var[:], AF.Ln, bias=eps_sb[:, 0:1])
            rinv = scr.tile([128, KC], F32, tag="rinv")
            nc.scalar.activation(rinv[:], lnv[:], AF.Exp, scale=-0.5)
            t1c = scr.tile([128, H, KC], F32, tag="fx1")
            for half, rp in ((0, rp0), (1, rp1)):
                nc.vector.tensor_tensor(
                    out=t1c[:, :, half * HC:(half + 1) * HC],
                    in0=_ap(rp[:], 0, [[1, H], [16, HC]]),
                    in1=_ap(rinv[:], half * HC, [[0, H], [1, HC]]), op=OP.mult)
            t2c = scr.tile([128, H, KC], F32, tag="fx2")
            nc.gpsimd.tensor_tensor(out=t2c[:], in0=t1c[:],
                                    in1=_ap(s_sb[:], ci * KC, [[L, H], [1, KC]]),
                                    op=OP.add)
            nc.scalar.activation(_ap(p_all[:], ci * KC, [[L, H], [1, KC]]),
                                 t2c[:], AF.Exp)

        pta = sing.tile([128, H, 2, 128], BF)
        for h in range(2 * NCH + 2):
            if h >= 2 and (h - 2) % 2 == 1:
                stageC((h - 2) // 2)
            if h == 12:
                # first half of every head's P^T: overlaps the rest of phase 1
                for hh in range(H):
                    nc.sync.dma_start_transpose(pta[:, hh, :, :], p_all[:, hh, 0:256])
            if h < 2 * NCH:
                stageA(h)
            if h >= 1 and h - 1 < 2 * NCH:
                stageB(h - 1)

        # ---------------- phase 2: PV (+denominator as ones-column), output ----------------
        pvps_full = ps.tile([128, L], F32, tag="p0")
        pvps = pvps_full[:, 0:264]
        for h in range(H):
            ptb = pvp.tile([128, 2, 128], BF, tag="pt")
            eng = nc.sync if h % 2 == 0 else nc.scalar
            eng.dma_start_transpose(ptb[:], p_all[:, h, 256:512])
            for kc4 in range(4):
                pt_sl = pta[:, h, kc4, :] if kc4 < 2 else ptb[:, kc4 - 2, :]
                nc.tensor.matmul(pvps[:, h * (DH + 1):(h + 1) * (DH + 1)],
                                 lhsT=pt_sl,
                                 rhs=v_sb[:, kc4, h * (DH + 1):(h + 1) * (DH + 1)],
                                 start=(kc4 == 0), stop=(kc4 == 3))

        denr = sing.tile([128, H], F32)
        nc.vector.tensor_scalar(out=denr[:], in0=_ap(pvps[:], DH, [[DH + 1, H]]),
                                scalar1=1e-30, scalar2=None, op0=OP.add)
        nc.vector.reciprocal(denr[:], denr[:])

        comb = sing.tile([128, D_IN], BF)
        t = scr.tile([128, D_IN], F32, tag="comb_t")
        nc.vector.tensor_tensor(out=t[:].rearrange("p (h d) -> p h d", h=H),
                                in0=_ap(pvps[:], 0, [[DH + 1, H], [1, DH]]),
                                in1=gate_sb[:].rearrange("p (h d) -> p h d", h=H),
                                op=OP.mult)
        nc.vector.tensor_tensor(out=comb[:].rearrange("p (h d) -> p h d", h=H),
                                in0=t[:].rearrange("p (h d) -> p h d", h=H),
                                in1=_ap(denr[:], 0, [[1, H], [0, DH]]), op=OP.mult)

        fin_full = ps.tile([128, L], F32, tag="p0")
        fin = fin_full[:, 0:D_IN]
        cT = pvp.tile([128, 2, 128], BF, tag="cT")
        nc.sync.dma_start_transpose(cT[:], comb[:])
        for c in range(2):
            nc.tensor.matmul(fin[:], lhsT=cT[:, c, :], rhs=w_sb["o"][:, c, :],
                             start=(c == 0), stop=False)
        nc.tensor.matmul(fin[:], lhsT=ones_row[:, 0:128], rhs=brow_sb[:, 4, :],
                         start=False, stop=True)
        out_sb = sing.tile([128, D_IN], F32)
        nc.scalar.activation(out_sb[:], fin[:], AF.Copy, scale=rowm_sb[:, 0:1])
        nc.sync.dma_start(out=out[:, :], in_=out_sb[:])

    # Steer insert_act_table_loads to the one set that covers Ln/Exp/Copy
    # (otherwise it alternates exp_and_others <-> natural_log, ~19 table loads).
    # Hiding functions from other sets only restricts choices; ids stay intact.
    orig_tables = bacc.get_activation_tables
    keep = "natural_log_exp_and_others"

    def _patched(arch):
        t = orig_tables(arch)
        return {name: (fs if name == keep else set()) for name, fs in t.items()}

    bacc.get_activation_tables = _patched
    try:
        nc.compile()
    finally:
        bacc.get_activation_tables = orig_tables
    return nc


def _prep_common(inputs):
    ln_in_g = np.asarray(inputs["ln_in_g"], np.float64)
    ln_in_b = np.asarray(inputs["ln_in_b"], np.float64)
    ln_b_g = np.asarray(inputs["ln_b_g"], np.float64)
    Wq = np.asarray(inputs["Wq"], np.float64)
    Wk = np.asarray(inputs["Wk"], np.float64)
    Wv = np.asarray(inputs["Wv"], np.float64)
    Wg = np.asarray(inputs["Wg"], np.float64)
    Wb = np.asarray(inputs["Wb"], np.float64)
    Wo = np.asarray(inputs["Wo"], np.float64)
    bg = np.asarray(inputs["bg"], np.float64)
    bo = np.asarray(inputs["bo"], np.float64)

    def arr_w(w):  # [256, 256] -> [128, 2, 256] din-chunk grouping
        return np.ascontiguousarray(
            w.reshape(2, 128, D_IN).transpose(1, 0, 2)).astype(BF16)

    wall = np.zeros((128, 12, D_IN), BF16)
    wall[:, 0:2] = arr_w(Wq * ln_in_g[:, None])
    wall[:, 2:4] = arr_w(Wk * ln_in_g[:, None] * SCALE)
    wall[:, 4:6] = arr_w(Wv * ln_in_g[:, None])
    wall[:, 6:8] = arr_w(Wg * ln_in_g[:, None])
    wall[:, 8:10] = arr_w(Wo)

    brows = np.stack([
        ln_in_b @ Wq,
        (ln_in_b @ Wk) * SCALE,
        ln_in_b @ Wv,
        ln_in_b @ Wg + bg,
        bo,
    ]).astype(BF16)

    c1 = ln_b_g @ Wb                        # [H]
    wext = np.zeros((D_BIAS, 16), np.float64)
    # head cols pre-centered: T @ (g*Wb - c1/128) == T@ (g*Wb) - mean(T)*c1
    wext[:, 0:H] = Wb * ln_b_g[:, None] - c1[None, :] / D_BIAS
    wext[:, 8] = 1.0 / D_BIAS
    wall[:, 10, 0:16] = wext.astype(BF16)
    wall[:, 11, 0:128] = np.eye(128, dtype=BF16)

    return dict(wall=wall, brows=brows)


def _make_in_maps(inputs):
    x = np.asarray(inputs["x"], np.float32)
    bias = np.asarray(inputs["bias"], np.float32)
    mask = np.asarray(inputs["mask"])
    common = _prep_common(inputs)

    in_maps = []
    for c in range(8):
        b, qb = divmod(c, 4)
        q0 = qb * QB
        rowm = np.zeros((128, 5), np.float32)
        rowm[:, 0] = (mask[b, q0:q0 + QB] != 0)
        rowm[:, 1:5] = (mask[b] != 0).astype(np.float32).reshape(4, 128).T
        nat = bias[b, q0:q0 + QB].astype(BF16)
        x_all = np.zeros((128, 5, D_IN), BF16)
        x_all[:, 0:4] = x[b].reshape(4, 128, D_IN).transpose(1, 0, 2)
        x_all[:, 4] = x[b, q0:q0 + QB]
        xfull = np.concatenate([x[b], x[b, q0:q0 + QB]], axis=0)   # [640, 256]
        x_allT = np.ascontiguousarray(
            xfull.T.reshape(2, 128, 640).transpose(1, 0, 2)).astype(BF16)
        in_maps.append(dict(
            bias_tr=np.ascontiguousarray(nat.transpose(2, 1, 0)),
            x_all=x_all, x_allT=x_allT,
            rowm=rowm,
            **common,
        ))
    return in_maps


def kernel(**inputs):
    if "nc" not in _CACHE:
        _CACHE["nc"] = _build()
    nc = _CACHE["nc"]

    in_maps = _make_in_maps(inputs)
    res = run_bass_kernel_spmd(nc, in_maps, list(range(8)))
    out = np.empty((B, L, D_IN), np.float32)
    for c in range(8):
        b, qb = divmod(c, 4)
        out[b, qb * QB:(qb + 1) * QB] = res.results[c]["out"]
    return out
